# revision 25
# baseline (speedup 1.0000x reference)
"""Trainium2 Bass kernel for nn_AttentionModel (4-layer dense transformer).

Contract: kernel(**inputs) takes FULL unsharded inputs (as produced by
setup_inputs) and returns the FULL output [N, L, V] fp32.

Sharding: data-parallel over batch N=8 across the 8 NeuronCores - each core
runs the complete transformer for one batch element (identical NEFF, per-core
tokens). No collectives needed; the host stacks the per-core outputs.

Per-core dataflow (L=1024, F=512, H=8, KD=QD=64, NL=4, V=1024):
  - embedding: indirect-DMA gather of f16 embed rows by token -> x0 [L, F]
  - activations: natural [l(128-part) x F] f16 (layernorm / residual /
    softmax scales) and transposed [F(128-part) x L] (matmul operands;
    fp8e4m3 for the Q/K/V projections, f32r for MLP/unembed). All natural
    tiles are f16 so the PE transposes run at 1.0 cyc/col.
  - per layer, scheduled COLUMN-MAJOR in two i-column phases (c=0: i<512,
    c=1: i>=512): each phase runs kv -> scores -> exp -> attend -> its own
    MLP half -> LN half -> y-transposes. Phase 0's outputs (xT8 blocks 0-3)
    are exactly what the NEXT layer's phase 0 consumes, so the next layer's
    score/exp stream starts while this layer's phase-1 MLP is still on PE.
  - kT/vT: fp8 DoubleRow projections drained to fp8 in a host-permuted
    column layout (head h's 64 dims at [32 partitions x 2 chunks]) so the
    K=64 score contractions are DR matmuls at 0.5 cyc/col. Wk/Wv are scaled
    by KV_ALPHA=8 on the host (keeps layer-0 k/v out of fp8 subnormals);
    exp's scale operand divides the score psum by ALPHA^2 for free.
  - q = x Wq fp8 DR, stored fp16 [j-chunk, head, 65] with a ones column so
    the attend matmuls also produce softmax row-sums.
  - att_u = exp(scores^T/ALPHA^2 - 5) fp16; diagonal tiles triangle-zeroed
    with one gpsimd affine_select per head-pair (keep j<=i).
  - attend: x_new[i-block, pair] = att_u^T @ [q | 1] (fp16); col 64 of each
    head = row-sum; reciprocal + broadcast multiply normalize on DVE.
  - MLP f32r; LN batched (bn_stats/aggr DVE, one [P,4] Ln/Exp rstd pair per
    half on ACT, applies on DVE); unembed f32r with one [128,1024] DMA per
    block.

Measured this session (8-core SPMD, on-device 1001-iter loop differencing,
solo process): 422-425 us, rel err 5.9e-3 (budget 2e-2); prior-session
baseline of this harness protocol: 453 us. Run-to-run variance across
processes is +-30-60 us, so all tuning decisions were made on solo
same-protocol runs; co-loading multiple variants in one process inflates
all of them by ~+70 us.

Negative results (measured, do not redo): ACT-routed layer-boundary
transpose drains +60 us (ACT head-of-line blocking); transposed-output
attend (one matmul per (jc, head) accumulating [65, i-cols] in psum,
x_newT produced directly, normalize via DRAM-roundtrip reciprocal
broadcast) +300 us - the long single-bank accumulation chains and the
16 DMA roundtrips/layer serialize the attend stream; with the normalize
ablated it is still +60 us vs this version. PSUM_CFG (2,2,1) (pa ring 2
at the cost of one pp2 slot) +21 us - the shared pp2 ring depth
dominates. MLP in fp8 DoubleRow (kernel_v6.py; includes the ones=1/8
trick so 8*x_new clears fp8 subnormals, relu descales via ACT
scale=0.125): +45 us and rel err 1.09e-2 - the error is dominated by
W1/W2 fp8 quantization, and DR stationary loads slow the MLP on hw just
as they did the scores. TRIMASK dve vs pool: no difference. LN applies
on Pool (LN_APPLY=pool) +190 us(!) - gpsimd ops carry ~us-scale
dispatch latency, fatal inside the exposed layer-boundary chain
(apply->transpose->kv->scores->exp); the Pool trimasks survive only
because their latency hides behind the exp stream. Interleaving pair-pairs
(0,1)/(2,3) so four score matmuls per jc hit four disjoint 32-row PE
strips and four banks back-to-back (kernel_v7.py): +7 us - PE row-tile
concurrency does not materialize for DoubleRow matmuls. Grouped att tiles
([P,4,2,512]) with ONE strided affine_select per (pair, phase) (ap
stride 1152, rank-3 zero-stride pattern) fails in neuronx-cc/walrus
codegen with an opaque PJRT error - per-tile masks are required, or the
AP/pattern needs a walrus-compatible formulation; a rank-2 per-head
variant compiles but crashes the exec unit on hw
(NRT_EXEC_UNIT_UNRECOVERABLE) - the 1152-stride gpsimd AP is illegal
on silicon (kernel_v5.py). Hw-vs-sim gap is
~100 us (sim 324 us): ablation probes put the wall spread across ALL
streams (exp cascade 53 us, scores 38, attend+norm 36, LN chain 37),
i.e. cross-engine overlap on silicon is much poorer than CoreSim
models - wider instructions or fewer sync hops help only if they do
not lengthen any single engine's serial chain.
"""

import numpy as np

import concourse.bass as bass
import concourse.mybir as mybir
import concourse.tile as tile
from concourse import bacc
from concourse.bass_utils import run_bass_kernel_spmd
from concourse.masks import make_identity, make_upper_triangular

# Model dims (hardcoded per the problem spec)
V, F, NL, H, KD, QD = 1024, 512, 4, 8, 64, 64
N, L = 8, 1024
HQ = H * QD  # 512
P = 128
FC = F // P      # 4 f-chunks
LB = L // P      # 8 l-blocks of 128
NCORES = 8

f32 = mybir.dt.float32
f32r = mybir.dt.float32r
f16 = mybir.dt.float16
f8 = mybir.dt.float8e4
i32 = mybir.dt.int32
AF = mybir.ActivationFunctionType
OP = mybir.AluOpType
DR = mybir.MatmulPerfMode.DoubleRow

_NC_CACHE: dict = {}
ABLATE = "none"  # perf-analysis knob: none|scores|attend|transposes
DR_MODE = "dr"  # q/k/v projection matmul mode: dr (fp8 DoubleRow, 256-deep
# K per pass) | fp8 (plain fp8, 128-deep chunks — isolates DoubleRow's real
# hw throughput from the fp8 layout changes)
SCORES_DR = True  # scores as fp8 DoubleRow: kT/vT drained to fp8 in a
# host-permuted column layout where head h's 64 dims sit at
# [32 partitions x 2 chunk-blocks], so the K=64 contraction runs as a DR
# matmul at 0.5 cyc/col (halves the scores' PE time). Wk/Wv are scaled by
# KV_ALPHA on the host (keeps layer-0 k/v out of fp8e4m3's subnormal range);
# the exp undoes alpha^2 via the ACT scale operand for free.
KV_ALPHA = 8.0
MLP_FP8 = False  # MLP1/MLP2 in fp8 DoubleRow (x_newT/h1T stored fp8);
# False keeps the f32r MLP path (measured faster on hw in-process A/B and
# halves the end-to-end error: 5.5e-3 vs 1.1e-2)
LN_BATCH = False  # batch the LN ln/exp across the 8 l-chunks
TCOPY = "dve"  # engine for merged y/x0 transpose copies: dve|act|split
# (674us vs 683us for act in same-process hw A/B)
TRIMASK = "pool"  # causal triangle zeroing of diagonal att tiles:
# dve (tensor_tensor multiply with a precomputed f16 mask, 4x mode, keeps
# the score->exp->attend cascade off the Pool engine) | pool (gpsimd
# affine_select, one fewer DVE op but an extra engine hop per diag tile)
EXPP_BUFS = 22  # in-flight fp16 att PAIR tiles ([P,2,512]); the interleaved
# schedule keeps pair p's 12 tiles live while pair p+1's 12 are produced
PSUM_CFG = (3, 1, 1)  # bufs for (pp2, pa, pt). pp2 tiles are [P,2,512]
# (2 banks, shared by scores pairs / projection pairs / mlp / unembed); pa
# packs 2 attend accumulators of 130 f32 into one bank; pt packs 4 transpose
# outputs into one bank. Banks: 3*2 + 1 + 1 = 8.


class _Bacc(bacc.Bacc):
    """Bacc with activation-table-set selection pinned to
    natural_log_exp_and_others (contains Exp, Ln, Relu, Copy — everything this
    kernel uses) so the load-insertion pass emits one table load instead of
    thrashing between per-function sets (~2.7us per swap)."""

    def insert_act_table_loads(self):
        from concourse.hw_specs import get_activation_tables
        import concourse.mybir as _mb

        has_activation = any(
            isinstance(i, _mb.InstActivation)
            for b in self.main_func.blocks
            for i in b.instructions
        )
        if not has_activation:
            return
        keep = {AF.Exp, AF.Ln, AF.Relu, AF.Copy}
        chosen = "natural_log_exp_and_others"
        full = get_activation_tables(self.m.arch)
        assert keep <= full[chosen], (chosen, keep - full[chosen])
        tables = [
            (name, (fns if name == chosen else fns - keep))
            for name, fns in full.items()
        ]
        import bass_rust as _bass_rust
        _bass_rust.insert_act_table_loads(self, tables)


LN_APPLY = "dve"  # engine for the LN applies: dve (4x-mode tensor_scalar,
# same queue as the stats so one cascade) | pool (slower per-op but skips
# the DVE queue backlog sitting ahead of the layer-boundary chain)


def _ln_apply(nc, y, b, mv8, rstd8, use_gamma, use_beta, gamma_b, beta_b):
    t = y[:, b, :]
    eng = nc.gpsimd if LN_APPLY == "pool" else nc.vector
    eng.tensor_scalar(
        t, t, mv8[:, b, 0:1], rstd8[:, b:b + 1],
        op0=OP.subtract, op1=OP.mult)
    if use_gamma:
        eng.tensor_mul(t, t, gamma_b[:])
    if use_beta:
        eng.tensor_add(t, t, beta_b[:])


def _r(ap):
    """View a DRAM fp32 AP as float32r for DMA into f32r tiles."""
    return ap.bitcast(f32r)


def _build(flags, repeat=1):
    use_b1, use_b2, use_gamma, use_beta, use_bout = flags
    nc = _Bacc("TRN2", target_bir_lowering=False, debug=False,
               num_devices=NCORES)

    tokens = nc.declare_dram_parameter("tokens", [L], i32, isOutput=False)
    embed = nc.declare_dram_parameter("embed16", [V, F], f16, isOutput=False)
    Wq8 = nc.declare_dram_parameter("Wq8", [NL, F, HQ], f8, isOutput=False)
    Wk8 = nc.declare_dram_parameter("Wk8", [NL, F, H * KD], f8, isOutput=False)
    Wv8 = nc.declare_dram_parameter("Wv8", [NL, F, H * KD], f8, isOutput=False)
    if MLP_FP8:
        W18 = nc.declare_dram_parameter("W18", [NL, HQ, F], f8, isOutput=False)
        W28 = nc.declare_dram_parameter("W28", [NL, F, F], f8, isOutput=False)
    W1 = nc.declare_dram_parameter("W1", [NL, HQ, F], f32, isOutput=False)
    b1 = nc.declare_dram_parameter("b1", [NL, F], f32, isOutput=False)
    W2 = nc.declare_dram_parameter("W2", [NL, F, F], f32, isOutput=False)
    b2 = nc.declare_dram_parameter("b2", [NL, F], f32, isOutput=False)
    gamma = nc.declare_dram_parameter("gamma", [NL, F], f32, isOutput=False)
    beta = nc.declare_dram_parameter("beta", [NL, F], f32, isOutput=False)
    Wout = nc.declare_dram_parameter("Wout", [F, V], f32, isOutput=False)
    bout = nc.declare_dram_parameter("bout", [V], f32, isOutput=False)
    out = nc.declare_dram_parameter("out", [L, V], f32, isOutput=True)

    with tile.TileContext(nc) as tc:
        with (
            tc.tile_pool(name="bigT", bufs=2) as bigT,    # [P, FC, L] f32r
            tc.tile_pool(name="kv8", bufs=2) as kv8p,     # [P, FC, L] fp8 k/v
            tc.tile_pool(name="t8", bufs=4 if MLP_FP8 else 2) as t8p,       # [P, FC, L] fp8
            tc.tile_pool(name="nat", bufs=3) as natp,     # [P, LB, F] f32
            tc.tile_pool(name="qp", bufs=1) as qp,        # [P, LB, H, 65] f16
            tc.tile_pool(name="expp", bufs=EXPP_BUFS) as expp,  # [P, 2, 512] f16
            tc.tile_pool(name="wp", bufs=4) as wp,
            tc.tile_pool(name="cst", bufs=1) as cst,
            tc.tile_pool(name="sm", bufs=16) as sm,       # small per-partition scalars
            tc.tile_pool(name="op", bufs=2) as outp,      # [P, 1024] out staging
            tc.tile_pool(name="pp2", bufs=PSUM_CFG[0], space="PSUM") as pp2,
            tc.tile_pool(name="pa", bufs=PSUM_CFG[1], space="PSUM") as pa,
            tc.tile_pool(name="pt", bufs=PSUM_CFG[2], space="PSUM") as pt,
        ):
            # ---- constants ----
            # f16 identity: every transpose source (x_nat/x_new/y) is f16,
            # and f16 PE transposes run at 1.0 cyc/col vs f32's 2.0
            ident = cst.tile([P, P], f16, tag="ident")
            make_identity(nc, ident[:])
            tri = cst.tile([P, P], f16, tag="tri")  # keep j<=i
            make_upper_triangular(nc, tri[:], val=1.0, diag=True)
            eps_t = cst.tile([P, 1], f32, tag="eps")
            nc.vector.memset(eps_t[:], 1e-5)
            neg5_t = cst.tile([P, 1], f32, tag="neg5")
            nc.vector.memset(neg5_t[:], -5.0)
            if use_b1:
                b1_sb = cst.tile([P, NL, FC], f32, tag="b1")
                nc.sync.dma_start(b1_sb[:], b1.rearrange("l (c p) -> p l c", p=P))
            if use_bout:
                bout_b = cst.tile([P, V], f32, tag="bout")
                bout_ap = bout[:]
                nc.sync.dma_start(
                    bout_b[:],
                    bass.AP(tensor=bout_ap.tensor, offset=bout_ap.offset,
                            ap=[[0, P]] + bout_ap.ap),
                )

            def bcast_row(dram_row_ap, tag):
                t = cst.tile([P, F], f32, tag=tag)
                nc.sync.dma_start(
                    t[:],
                    bass.AP(tensor=dram_row_ap.tensor, offset=dram_row_ap.offset,
                            ap=[[0, P]] + dram_row_ap.ap),
                )
                return t

            import contextlib
            static_et: list = []  # ablation: constant att tiles
            _loop = (tc.For_i(0, repeat, 1) if repeat > 1
                     else contextlib.nullcontext())
            with _loop:
                # ---- embedding gather ----
                tok_sb = cst.tile([P, LB], i32, tag="tok")
                nc.sync.dma_start(tok_sb[:], tokens.rearrange("(b p) -> p b", p=P))
                x_nat = natp.tile([P, LB, F], f16, tag="nat")
                if ABLATE == "embed":
                    nc.gpsimd.memset(x_nat[:], 0.02)
                else:
                    for b in range(LB):
                        nc.gpsimd.indirect_dma_start(
                            out=x_nat[:, b, :], out_offset=None,
                            in_=embed[:],
                            in_offset=bass.IndirectOffsetOnAxis(ap=tok_sb[:, b:b + 1], axis=0),
                        )

                def tcopy(i, dst, src):
                    """Merged psum->sbuf copy; engine per TCOPY."""
                    if TCOPY == "dve" or (TCOPY == "split" and i % 2 == 0):
                        nc.vector.tensor_copy(dst, src)
                    else:
                        nc.scalar.copy(dst, src)

                def transpose_blocks(src_nat, dst_T, b0, nb, eng=None):
                    """Transpose l-blocks [b0, b0+nb) of natural [P, LB, F]
                    f16 into T layout [P, FC, L] (dtype cast per dst tile:
                    f32r or fp8). 4 transpose outputs share one psum bank and
                    drain with a single merged copy (engine overridable: the
                    layer-boundary half-0 drains go to ACT so the
                    next-layer-critical chain never enters the DVE FIFO)."""
                    if ABLATE == "transposes":
                        nc.gpsimd.memset(dst_T[:, :, b0 * P:(b0 + nb) * P], 0.1)
                        return
                    for b in range(b0, b0 + nb):
                        pt_ps = pt.tile([P, FC, P], f16, tag="pt")
                        for c in range(FC):
                            nc.tensor.transpose(
                                pt_ps[:, c, :],
                                src_nat[:, b, c * P:(c + 1) * P], ident[:])
                        if eng is not None:
                            eng(dst_T[:, :, b * P:(b + 1) * P], pt_ps[:])
                        else:
                            tcopy(b, dst_T[:, :, b * P:(b + 1) * P], pt_ps[:])

                xT8 = t8p.tile([P, FC, L], f8, tag="t8")
                # blocks 0-3 only: layer 0's first K/V half (and the first
                # scores) need just these; blocks 4-7 are emitted inside the
                # layer-0 head so kv0-lc0 isn't queued behind transposes
                # whose gathers land ~3us later
                transpose_blocks(x_nat, xT8, 0, 4)

                # ---- layers ----
                for li in range(NL):
                    last = li == NL - 1
                    wq8_t = wp.tile([P, FC, HQ], f8, tag="w8", bufs=5 if MLP_FP8 else 3)
                    wk8_t = wp.tile([P, FC, HQ], f8, tag="w8", bufs=5 if MLP_FP8 else 3)
                    wv8_t = wp.tile([P, FC, HQ], f8, tag="w8", bufs=5 if MLP_FP8 else 3)
                    if ABLATE != "wdma":
                        # consumption order: wk8/wv8 gate the layer's first
                        # K/V chunk; wq8 isn't read until the first scores
                        # are already in flight
                        nc.sync.dma_start(wk8_t[:], Wk8[li].rearrange("(c p) o -> p c o", p=P))
                        nc.sync.dma_start(wv8_t[:], Wv8[li].rearrange("(c p) o -> p c o", p=P))
                        nc.sync.dma_start(wq8_t[:], Wq8[li].rearrange("(c p) o -> p c o", p=P))
                    if MLP_FP8:
                        w1_t = wp.tile([P, FC, F], f8, tag="w8", bufs=5 if MLP_FP8 else 3)
                        w2_t = wp.tile([P, FC, F], f8, tag="w8", bufs=5 if MLP_FP8 else 3)
                        nc.sync.dma_start(
                            w1_t[:], W18[li].rearrange("(c p) o -> p c o", p=P))
                        nc.sync.dma_start(
                            w2_t[:], W28[li].rearrange("(c p) o -> p c o", p=P))
                    else:
                        w1_t = wp.tile([P, FC, F], f32r, tag="w", bufs=4)
                        w2_t = wp.tile([P, FC, F], f32r, tag="w", bufs=4)
                        if ABLATE != "wdma":
                            nc.sync.dma_start(
                                w1_t[:], _r(W1[li].rearrange("(c p) o -> p c o", p=P)))
                            nc.sync.dma_start(
                                w2_t[:], _r(W2[li].rearrange("(c p) o -> p c o", p=P)))

                    # kT, vT: fp8 (scores are DR matmuls reading them with
                    # head h's 64 dims at [32 partitions x 2 chunks] — the
                    # host permutes Wk/Wv columns into that layout). Emission
                    # granularity (chunk-pair c2, l-half lc): pairs 0,1 of
                    # scores need chunks 0-1, pairs 2,3 need chunks 2-3; the
                    # lc0 halves need only xT8 blocks 0-3 (ready right after
                    # the previous layer's early half-0 LN).
                    kT = kv8p.tile([P, FC, L], f8, tag="kv8")
                    vT = kv8p.tile([P, FC, L], f8, tag="kv8")

                    def emit_kv2(c2, lc, eng=None):
                        for w8_t, oT in ((wk8_t, kT), (wv8_t, vT)):
                            ps = pp2.tile([P, 2, 512], f32, tag="pp2")
                            for ch in range(2):
                                oc = 2 * c2 + ch
                                for fc2 in range(0, FC, 2):
                                    nc.tensor.matmul(
                                        ps[:, ch, :],
                                        w8_t[:, fc2:fc2 + 2, oc * P:(oc + 1) * P],
                                        xT8[:, fc2:fc2 + 2, lc * 512:(lc + 1) * 512],
                                        start=(fc2 == 0), stop=(fc2 == FC - 2),
                                        perf_mode=DR)
                            (eng or nc.vector.tensor_copy)(
                                oT[:, 2 * c2:2 * c2 + 2, lc * 512:(lc + 1) * 512],
                                ps[:])

                    # q natural (fp16 for the attend matmul), [P(j), jc, head, 65]
                    # with a trailing ones column so attend also yields row-sums
                    q_sb = qp.tile([P, LB, H, 65], f16, tag="q")
                    # ones column on Pool: a DVE memset here would
                    # head-of-line-block the next layer's kv drains behind
                    # this tile's wait on the previous layer's last attend
                    nc.gpsimd.memset(q_sb[:, :, :, 64:65], 1.0)

                    def emit_q2(bp):
                        ps = pp2.tile([P, 2, 512], f32, tag="pp2")
                        for i2 in range(2):
                            b = 2 * bp + i2
                            if DR_MODE == "dr":
                                for fc2 in range(0, FC, 2):
                                    nc.tensor.matmul(
                                        ps[:, i2, :],
                                        xT8[:, fc2:fc2 + 2, b * P:(b + 1) * P],
                                        wq8_t[:, fc2:fc2 + 2, :],
                                        start=(fc2 == 0), stop=(fc2 == FC - 2),
                                        perf_mode=DR)
                            else:
                                for fc in range(FC):
                                    nc.tensor.matmul(
                                        ps[:, i2, :],
                                        xT8[:, fc, b * P:(b + 1) * P],
                                        wq8_t[:, fc, :],
                                        start=(fc == 0), stop=(fc == FC - 1))
                        eng = nc.vector.tensor_copy if bp % 2 else nc.scalar.copy
                        eng(q_sb[:, 2 * bp:2 * bp + 2, :, 0:64],
                            ps[:].rearrange("p b (h d) -> p b h d", h=H))

                    x_new = natp.tile([P, LB, F], f16, tag="nat")
                    if MLP_FP8:
                        x_newT = t8p.tile([P, FC, L], f8, tag="t8")
                    else:
                        x_newT = bigT.tile([P, FC, L], f32r, tag="bigT")
                    exp_store: dict = {}

                    def emit_xnewT(p, half):
                        # transposes of x_new chunk p (head pair p's columns)
                        # for the 4 l-blocks finished by attend(p, half).
                        # Pair 3's copies gate MLP1 and run when the exps are
                        # done, so they drain on the then-idle ACT.
                        pt_ps = pt.tile([P, 4, P], f16, tag="pt")
                        for i, b in enumerate(range(4 * half, 4 * half + 4)):
                            nc.tensor.transpose(
                                pt_ps[:, i, :],
                                x_new[:, b, p * P:(p + 1) * P], ident[:])
                        eng = nc.scalar.copy if p == 3 else nc.vector.tensor_copy
                        eng(x_newT[:, p, 4 * half * P:(4 * half + 4) * P],
                            pt_ps[:])

                    def emit_scores(hpair, c):
                        # DR scores: head h operand = [32 partitions, 2
                        # chunks] of the permuted k/v layout; exp's scale
                        # undoes the host-side KV_ALPHA^2.
                        # Ablation probes (attend reads constant tiles so the
                        # scores->exp->attend cascade is truly severed):
                        #   scores: no matmul, no exp  |  exp: matmul only
                        hp_base = 64 * (hpair % 2)
                        cc = 2 * (hpair // 2)
                        ablated = ABLATE in ("scores", "exp", "skeleton")
                        if ablated and not static_et:
                            for _ in range(LB):
                                t = expp.tile([P, 2, 512], f16, tag="exp")
                                nc.gpsimd.memset(t[:], 0.5)
                                static_et.append(t)
                        tiles = {}
                        for jc in range(4 * c + 4):
                            d = jc - 4 * c
                            n0 = 0 if d < 0 else min(P * d, 256)
                            e0 = 0 if d < 0 else P * d
                            # both heads of the pair in one 2-bank psum tile
                            # so exp / affine_select run as single wide
                            # instructions (halves the per-instr ACT access
                            # latency spend)
                            if ABLATE not in ("scores", "skeleton"):
                                ps = pp2.tile([P, 2, 512], f32, tag="pp2")
                                for hi in range(2):
                                    hp0 = hp_base + 32 * hi
                                    nc.tensor.matmul(
                                        ps[:, hi, n0:512],
                                        vT[hp0:hp0 + 32, cc:cc + 2, jc * P:(jc + 1) * P],
                                        kT[hp0:hp0 + 32, cc:cc + 2, c * 512 + n0:(c + 1) * 512],
                                        start=True, stop=True, perf_mode=DR,
                                        tile_position=(hp0, 0))
                            if ablated:
                                tiles[jc] = static_et[jc]
                                continue
                            et = expp.tile([P, 2, 512], f16, tag="exp")
                            # bias=-5: softmax is shift-invariant (both the
                            # attend numerator and the ones-column row-sum
                            # scale by e^-5), keeps exp within fp16 range
                            nc.scalar.activation(
                                et[:, :, e0:512], ps[:, :, e0:512], AF.Exp,
                                bias=neg5_t[:],
                                scale=1.0 / (KV_ALPHA * KV_ALPHA))
                            if d >= 0 and ABLATE != "trimask":
                                # zero att where j > i, both heads at once
                                if TRIMASK == "dve":
                                    nc.vector.tensor_tensor(
                                        et[:, :, e0:e0 + P],
                                        et[:, :, e0:e0 + P],
                                        tri[:, None, :].to_broadcast(
                                            (P, 2, P)),
                                        OP.mult)
                                else:
                                    nc.gpsimd.affine_select(
                                        out=et[:, :, e0:e0 + P],
                                        in_=et[:, :, e0:e0 + P],
                                        compare_op=OP.is_ge,
                                        fill=0.0, base=0,
                                        pattern=[[0, 2], [1, P]],
                                        channel_multiplier=-1)
                            tiles[jc] = et
                        exp_store[(hpair, c)] = tiles

                    def emit_attend(hpair, c):
                        heads = (2 * hpair, 2 * hpair + 1)
                        tiles = exp_store.pop((hpair, c))
                        # two b-slots share one psum bank (2 x 130 f32);
                        # normalization is batched per b-pair: one strided
                        # reciprocal + one broadcast multiply for both slots
                        pa_t = pa.tile([P, 2, 130], f32, tag="pa")
                        for b0 in range(4 * c, 4 * c + 4, 2):
                            if ABLATE in ("attend", "skeleton"):
                                for h in heads:
                                    nc.gpsimd.memset(
                                        x_new[:, b0:b0 + 2,
                                              h * 64:(h + 1) * 64], 0.1)
                                continue
                            for s, b in enumerate((b0, b0 + 1)):
                                lc0 = (b - 4 * c) * P
                                # both heads of the pair accumulate into one
                                # psum bank: head h' at cols [65*h', 65*h'+65)
                                for hi, h in enumerate(heads):
                                    for jc in range(b + 1):
                                        nc.tensor.matmul(
                                            pa_t[:, s, 65 * hi:65 * hi + 65],
                                            tiles[jc][:, hi, lc0:lc0 + P],
                                            q_sb[:, jc, h, :],
                                            start=(jc == 0), stop=(jc == b))
                            if ABLATE in ("norm",):
                                nc.gpsimd.memset(
                                    x_new[:, b0:b0 + 2,
                                          hpair * P:(hpair + 1) * P], 0.1)
                                continue
                            pa4 = pa_t[:].rearrange("p s (h x) -> p s h x", h=2)
                            rc = sm.tile([P, 2, 2], f32, tag="rc")
                            nc.vector.reciprocal(rc[:], pa4[:, :, :, 64])
                            # x_new[:, b0:b0+2, pair] = att_u @ q * recip
                            # (recip broadcast 64-wide per head, 0-stride)
                            xdst = x_new[:, b0:b0 + 2,
                                         hpair * P:(hpair + 1) * P].rearrange(
                                "p b (h x) -> p b h x", h=2)
                            nc.vector.tensor_tensor(
                                xdst, pa4[:, :, :, 0:64],
                                rc[:, :, :, None].to_broadcast((P, 2, 2, 64)),
                                OP.mult)

                    # MLP tiles & LN state up front: the c-major schedule
                    # interleaves the MLP halves with the attention column
                    # phases, so these are referenced mid-attention
                    h1T = bigT.tile([P, FC, L], f32r, tag="bigT")
                    if use_b2:
                        b2_b = bcast_row(b2[li], f"b2_{li}")
                    if use_gamma:
                        gamma_b = bcast_row(gamma[li], f"g_{li}")
                    if use_beta:
                        beta_b = bcast_row(beta[li], f"be_{li}")
                    if last:
                        wo = []
                        for vc in range(2):
                            wt = wp.tile([P, FC, 512], f32r, tag="w", bufs=4)
                            nc.sync.dma_start(
                                wt[:],
                                _r(Wout[:, vc * 512:(vc + 1) * 512]
                                   .rearrange("(c p) o -> p c o", p=P)))
                            wo.append(wt)
                        xT_next = bigT.tile([P, FC, L], f32r, tag="bigT")
                    else:
                        xT_next = t8p.tile([P, FC, L], f8, tag="t8")
                    y = natp.tile([P, LB, F], f16, tag="nat")
                    mv8 = sm.tile([P, LB, 2], f32, tag="mv8")
                    rstd8 = sm.tile([P, LB], f32, tag="rs8")

                    def emit_unembed(b):
                        # fc outer / vc inner: consecutive matmuls share the
                        # stationary xT block -> one ldweights per fc
                        ps = pp2.tile([P, 2, 512], f32, tag="pp2")
                        for fc in range(FC):
                            for vc in range(2):
                                nc.tensor.matmul(
                                    ps[:, vc, :],
                                    xT_next[:, fc, b * P:(b + 1) * P],
                                    wo[vc][:, fc, :],
                                    start=(fc == 0), stop=(fc == FC - 1))
                        if ABLATE == "outdma":
                            return
                        ot = outp.tile([P, V], f32, tag="o")
                        psf = ps[:].rearrange("p a b -> p (a b)")
                        if use_bout:
                            nc.vector.tensor_add(ot[:], psf, bout_b[:])
                        else:
                            eng = (nc.vector.tensor_copy if b % 2
                                   else nc.scalar.copy)
                            eng(ot[:], psf)
                        nc.sync.dma_start(out[b * P:(b + 1) * P, :], ot[:])

                    def emit_ln_half(half):
                        # batched rstd = exp(-0.5*ln(var+eps)), 4 blocks per
                        # [P,4] ACT pair instead of 16 tiny per-block ops —
                        # the hw charges ~us-scale latency per cross-engine
                        # cascade (measured ~210us total for the per-block
                        # version via ablation). Half 0 is emitted mid-MLP2
                        # (after bp=1) so its applies/transposes don't queue
                        # behind blocks 4-7's stats on the in-order DVE/PE
                        # queues: the next layer's first K/V chunk only needs
                        # blocks 0-3 transposed.
                        h0 = 4 * half
                        if ABLATE not in ("ln", "skeleton"):
                            nc.scalar.activation(
                                rstd8[:, h0:h0 + 4], mv8[:, h0:h0 + 4, 1],
                                AF.Ln, bias=eps_t[:])
                            nc.scalar.activation(
                                rstd8[:, h0:h0 + 4], rstd8[:, h0:h0 + 4],
                                AF.Exp, scale=-0.5)
                            for b in range(h0, h0 + 4):
                                _ln_apply(nc, y, b, mv8, rstd8, use_gamma,
                                          use_beta,
                                          gamma_b if use_gamma else None,
                                          beta_b if use_beta else None)
                        # (routing these drains to ACT was tried to shorten
                        # the layer-boundary chain; it measured +60us on hw —
                        # ACT head-of-line blocking outweighs the DVE queue)
                        transpose_blocks(y, xT_next, h0, 4)
                        if last:
                            for b in range(h0, h0 + 4):
                                emit_unembed(b)

                    def mlp1_half(h):
                        # MLP1 for i-column half h: h1T[:, :, h*512:(h+1)*512]
                        # = relu(W1^T x_newT-half); two out-chunks share one
                        # [P,2,512] psum pair and one wide relu drain
                        for ocp in range(2):
                            ps = pp2.tile([P, 2, 512], f32, tag="pp2")
                            for i2 in range(2):
                                oc = 2 * ocp + i2
                                for fc in range(FC):
                                    nc.tensor.matmul(
                                        ps[:, i2, :],
                                        w1_t[:, fc, oc * P:(oc + 1) * P],
                                        x_newT[:, fc, h * 512:(h + 1) * 512],
                                        start=(fc == 0), stop=(fc == FC - 1))
                            if use_b1:
                                for i2 in range(2):
                                    oc = 2 * ocp + i2
                                    nc.scalar.activation(
                                        h1T[:, oc, h * 512:(h + 1) * 512],
                                        ps[:, i2, :], AF.Relu,
                                        bias=b1_sb[:, li, oc:oc + 1])
                            else:
                                nc.scalar.activation(
                                    h1T[:, 2 * ocp:2 * ocp + 2,
                                        h * 512:(h + 1) * 512],
                                    ps[:], AF.Relu)

                    def mlp2_bp(bp):
                        ps = pp2.tile([P, 2, 512], f32, tag="pp2")
                        for i2 in range(2):
                            b = 2 * bp + i2
                            for fc in range(FC):
                                nc.tensor.matmul(
                                    ps[:, i2, :],
                                    h1T[:, fc, b * P:(b + 1) * P],
                                    w2_t[:, fc, :],
                                    start=(fc == 0), stop=(fc == FC - 1))
                        t2 = y[:, 2 * bp:2 * bp + 2, :]
                        nc.vector.tensor_add(t2, ps[:], x_nat[:, 2 * bp:2 * bp + 2, :])
                        if use_b2:
                            nc.vector.tensor_add(
                                t2, t2,
                                b2_b[:, None, :].to_broadcast((P, 2, F)))
                        if ABLATE not in ("ln", "skeleton"):
                            for i2 in range(2):
                                b = 2 * bp + i2
                                st = sm.tile([P, 6], f32, tag="st")
                                nc.vector.bn_stats(st[:], y[:, b, :])
                                nc.vector.bn_aggr(mv8[:, b, :], st[:])

                    # ---- c-major schedule ----
                    # Phase c=0 (i-cols 0-511) needs only kv lc0 + q blocks
                    # 0-3; its MLP half + LN + transposes complete xT8'
                    # blocks 0-3 — everything the NEXT layer's phase 0
                    # needs — so the next layer's exp stream starts while
                    # this layer's phase-1 MLP is still on PE. ACT sees an
                    # (almost) gap-free exp queue:
                    #   exps(p,0) x4, exps(0..3,1), relus, ln rstd, next layer
                    emit_kv2(0, 0, eng=nc.scalar.copy)
                    emit_scores(0, 0)
                    if li == 0:
                        transpose_blocks(x_nat, xT8, 4, 4)
                    emit_q2(0)
                    emit_q2(1)
                    emit_scores(1, 0)
                    emit_kv2(1, 0)
                    emit_attend(0, 0)
                    emit_xnewT(0, 0)
                    emit_scores(2, 0)
                    emit_attend(1, 0)
                    emit_xnewT(1, 0)
                    emit_scores(3, 0)
                    emit_attend(2, 0)
                    emit_xnewT(2, 0)
                    # phase-1 operands while the phase-0 exps drain
                    emit_kv2(0, 1)
                    emit_kv2(1, 1)
                    emit_q2(2)
                    emit_q2(3)
                    emit_scores(0, 1)
                    emit_attend(3, 0)
                    emit_xnewT(3, 0)
                    emit_scores(1, 1)
                    emit_attend(0, 1)
                    emit_xnewT(0, 1)
                    # phase-0 MLP under the phase-1 exp stream
                    mlp1_half(0)
                    emit_scores(2, 1)
                    mlp2_bp(0)
                    mlp2_bp(1)
                    emit_attend(1, 1)
                    emit_xnewT(1, 1)
                    emit_scores(3, 1)
                    if not last:
                        # half-0 LN + transposes release the next layer
                        # (on the last layer this must wait for mlp1_half(1):
                        # xT_next shares x_newT's bigT ring slot, so its
                        # writes would deadlock behind unembed in PE order)
                        emit_ln_half(0)
                    emit_attend(2, 1)
                    emit_xnewT(2, 1)
                    emit_attend(3, 1)
                    emit_xnewT(3, 1)
                    mlp1_half(1)
                    if last:
                        emit_ln_half(0)
                    mlp2_bp(2)
                    mlp2_bp(3)
                    emit_ln_half(1)

                    x_nat = y
                    xT8 = xT_next
    nc.compile()
    return nc


def _get_nc(flags, repeat=1):
    key = (flags, repeat, ABLATE, LN_BATCH, PSUM_CFG, TCOPY, EXPP_BUFS,
           DR_MODE, MLP_FP8, TRIMASK, LN_APPLY)
    if key not in _NC_CACHE:
        _NC_CACHE[key] = _build(flags, repeat)
    return _NC_CACHE[key]


def make_runner(flags, in_maps, repeat=1):
    """Build a reusable jitted SPMD runner with device-resident inputs.

    Returns (run, split_outputs) where run() executes the kernel once on all
    8 cores and blocks; used by test.py for timing without per-call host->device
    input transfer.
    """
    import jax
    from jax.sharding import Mesh, PartitionSpec, NamedSharding
    from concourse import bass2jax, mybir as _mybir

    bass2jax.install_neuronx_cc_hook()
    nc = _get_nc(flags, repeat)
    partition_name = (nc.partition_id_tensor.name if nc.partition_id_tensor
                      else None)
    in_names, out_names, out_avals, zero_outs = [], [], [], []
    for alloc in nc.m.functions[0].allocations:
        if not isinstance(alloc, _mybir.MemoryLocationSet):
            continue
        name = alloc.memorylocations[0].name
        if alloc.kind == "ExternalInput":
            if name != partition_name:
                in_names.append(name)
        elif alloc.kind == "ExternalOutput":
            shape = tuple(alloc.tensor_shape)
            dtype = _mybir.dt.np(alloc.dtype)
            out_names.append(name)
            out_avals.append(jax.core.ShapedArray(shape, dtype))
            zero_outs.append(np.zeros(shape, dtype))
    n_params = len(in_names)
    n_outs = len(out_avals)
    all_names = in_names + out_names + ([partition_name] if partition_name else [])

    def _body(*args):
        operands = list(args)
        if partition_name is not None:
            operands.append(bass2jax.partition_id_tensor())
        outs = bass2jax._bass_exec_p.bind(
            *operands,
            out_avals=tuple(out_avals),
            in_names=tuple(all_names),
            out_names=tuple(out_names),
            lowering_input_output_aliases=(),
            sim_require_finite=True,
            sim_require_nnan=True,
            nc=nc,
        )
        return tuple(outs)

    from jax.experimental.shard_map import shard_map
    devices = jax.devices()[:NCORES]
    mesh = Mesh(np.asarray(devices), ("core",))
    in_specs = (PartitionSpec("core"),) * (n_params + n_outs)
    out_specs = (PartitionSpec("core"),) * n_outs
    sharded = jax.jit(
        shard_map(_body, mesh=mesh, in_specs=in_specs, out_specs=out_specs,
                  check_rep=False),
        keep_unused=True,
    )
    concat_in = [
        np.concatenate([np.asarray(in_maps[c][nm])[None] for c in range(NCORES)],
                       axis=0).reshape(NCORES * np.asarray(in_maps[0][nm]).shape[0],
                                       *np.asarray(in_maps[0][nm]).shape[1:])
        for nm in in_names
    ]
    sh = NamedSharding(mesh, PartitionSpec("core"))
    dev_in = [jax.device_put(x, sh) for x in concat_in]
    dev_zeros = [
        jax.device_put(np.zeros((NCORES * z.shape[0], *z.shape[1:]), z.dtype), sh)
        for z in zero_outs
    ]

    def run():
        outs = sharded(*dev_in, *dev_zeros)
        jax.block_until_ready(outs)
        return outs

    def split(outs):
        return [
            {nm: np.asarray(outs[i]).reshape(NCORES, *out_avals[i].shape)[c]
             for i, nm in enumerate(out_names)}
            for c in range(NCORES)
        ]

    return run, split


def _kv_perm():
    """Column permutation mapping feature (h, d) -> 256*(h//4) + 128*(d//32)
    + 32*(h%4) + (d%32): head h's 64 contraction dims land at partitions
    [32*(h%4), +32) of chunks (2*(h//4), +2), the [32p x 2-chunk] layout the
    DR score matmuls consume. Score k.v is permutation-invariant as long as
    Wk and Wv share the permutation."""
    src = np.empty(HQ, dtype=np.int64)
    for h in range(H):
        for d in range(KD):
            src[256 * (h // 4) + 128 * (d // 32) + 32 * (h % 4) + (d % 32)] = \
                h * KD + d
    return src


def prep_args(inputs):
    """Host-side arg prep shared by kernel() and test.py: fp32 copies of the
    fp32 params, fp8e4m3 casts of Wq/Wk/Wv (consumed by the DoubleRow
    projection matmuls). With SCORES_DR, Wk/Wv columns are permuted to the
    DR score layout and scaled by KV_ALPHA (so layer-0 k/v values, ~0.009,
    clear fp8e4m3's subnormal floor when kT/vT are re-quantized to fp8; the
    exp's scale operand divides the score psum by KV_ALPHA^2)."""
    import ml_dtypes
    args = {k: np.ascontiguousarray(np.asarray(v), dtype=np.float32)
            for k, v in inputs.items()
            if k not in ("tokens", "Wq", "Wk", "Wv", "embed")}
    args["embed16"] = np.ascontiguousarray(
        np.asarray(inputs["embed"], dtype=np.float32).astype(np.float16))
    args["Wq8"] = np.ascontiguousarray(
        np.asarray(inputs["Wq"], dtype=np.float32).astype(ml_dtypes.float8_e4m3))
    perm = _kv_perm() if SCORES_DR else np.arange(HQ)
    for k in ("Wk", "Wv"):
        w = np.asarray(inputs[k], dtype=np.float32)
        if SCORES_DR:
            w = (w * KV_ALPHA)[:, :, perm]
        args[k + "8"] = np.ascontiguousarray(w.astype(ml_dtypes.float8_e4m3))
    if MLP_FP8:
        for k in ("W1", "W2"):
            args[k + "8"] = np.ascontiguousarray(
                args[k].astype(ml_dtypes.float8_e4m3))
    return args


def kernel(**inputs) -> np.ndarray:
    tokens = np.asarray(inputs["tokens"])
    args = prep_args(inputs)
    flags = (
        bool(np.any(args["b1"])),
        bool(np.any(args["b2"])),
        bool(np.any(args["gamma"] != 1.0)),
        bool(np.any(args["beta"])),
        bool(np.any(args["bout"])),
    )
    nc = _get_nc(flags)
    tok32 = np.ascontiguousarray(tokens.astype(np.int32))
    in_maps = [dict(args, tokens=tok32[c]) for c in range(NCORES)]
    res = run_bass_kernel_spmd(nc, in_maps, list(range(NCORES)))
    return np.stack([res.results[c]["out"] for c in range(NCORES)], axis=0)


if __name__ == "__main__":
    rng = np.random.default_rng(0)
    toy = {
        "tokens": rng.integers(0, V, size=(N, L)),
        "embed": rng.standard_normal((V, F)).astype(np.float32) * 0.02,
        "Wq": rng.standard_normal((NL, F, HQ)).astype(np.float32) * 0.02,
        "Wk": rng.standard_normal((NL, F, H * KD)).astype(np.float32) * 0.02,
        "Wv": rng.standard_normal((NL, F, H * KD)).astype(np.float32) * 0.02,
        "W1": rng.standard_normal((NL, HQ, F)).astype(np.float32) * 0.02,
        "b1": np.zeros((NL, F), np.float32),
        "W2": rng.standard_normal((NL, F, F)).astype(np.float32) * 0.02,
        "b2": np.zeros((NL, F), np.float32),
        "gamma": np.ones((NL, F), np.float32),
        "beta": np.zeros((NL, F), np.float32),
        "Wout": rng.standard_normal((F, V)).astype(np.float32) * 0.02,
        "bout": np.zeros((V,), np.float32),
    }
    o = kernel(**toy)
    print("out:", o.shape, o.dtype, float(np.abs(o).max()))

